# revision 33
# baseline (speedup 1.0000x reference)
"""Trainium2 Bass kernel for nn_CoarseView (gnn_message_passing).

Self-contained: host shards FULL inputs across 8 NeuronCores, runs one
SPMD Bass/Tile program (adapters -> gather xW1 -> adj matmuls with
collective gathers between stages -> global LN + row-norm), gathers the
FULL output.

Sharding: 1D row partition of the adjacency matmuls (978 rows/core,
core 7 zero-padded). Adjacency row-shards are transposed on the host so
the contraction index lands on SBUF partitions with contiguous DMAs, and
cast to bf16 to halve the ~2 GB HBM stream. Adapter/GCN weights are
replicated; the 2-layer adjacency mean is folded into 0.5*W1 / 0.5*W2.
"""

import sys
import types

import numpy as np
import ml_dtypes

# ---- NTFF profile hook shim (this image's antenv lacks axon_hooks; the
# boot-time registration degrades silently without it). Harmless if unused.
def _install_axon_hook_shim():
    try:
        import antenv  # noqa: F401
    except Exception:
        return
    if "antenv.axon_hooks" in sys.modules:
        return
    mod = types.ModuleType("antenv.axon_hooks")
    _h = {"hook": None}
    mod.set_axon_ntff_profile_hook = lambda h: _h.__setitem__("hook", h)
    mod.get_axon_ntff_profile_hook = lambda: _h["hook"]
    sys.modules["antenv.axon_hooks"] = mod
    try:
        import antenv as _a

        _a.axon_hooks = mod
        from trn_agent_boot.trn_boot import _ntff_profile_via_ctypes

        hook = _ntff_profile_via_ctypes("/opt/axon/libaxon_pjrt.so")
        mod.set_axon_ntff_profile_hook(hook)
    except Exception:
        pass


_install_axon_hook_shim()

import concourse.bacc as bacc
import concourse.tile as tile
from concourse import mybir
from concourse import bass_utils
from concourse.masks import make_identity

F32 = mybir.dt.float32
BF16 = mybir.dt.bfloat16
F8 = mybir.dt.float8e4
AF = mybir.ActivationFunctionType
ALU = mybir.AluOpType

N_CORES = 8
N_NODES = 7823
N_DRUG = 708
N_TARGET = 1512
FEAT = 1024
HID = 512
OUT = 256
LN_EPS = 1e-5

R = 978  # adjacency rows per core (core 7: 977 real + 1 zero pad)
PAD_N = 7936  # 62 * 128, padded contraction length
NT = PAD_N // 128  # 62 K-tiles
SEG = 1152  # 9 * 128, padded adapter segment rows
SEG_OFF = [0, 708, 1464, 2220, 3341, 4462, 5583, 6704]
SEG_VALID = [708, 756, 756, 1121, 1121, 1121, 1121, 1119]
SEG_TYPE = [0, 1, 1, 2, 2, 2, 2, 2]  # 0=drug(dA) 1=target(tA) 2=disease(sA)
NTOT = N_NODES * OUT  # global-LN element count

_PROGRAM = {"nc": None}


def _build_program():
    nc = bacc.Bacc(
        "TRN2",
        target_bir_lowering=False,
        debug=False,
        enable_asserts=False,
        num_devices=N_CORES,
    )

    # ---- I/O ----
    xsegT = nc.dram_tensor("xsegT", [FEAT, SEG], BF16, kind="ExternalInput").ap()
    aWT = nc.dram_tensor("aWT", [FEAT, HID], BF16, kind="ExternalInput").ap()
    aWrT = nc.dram_tensor("aWrT", [FEAT, HID], BF16, kind="ExternalInput").ap()
    ab = nc.dram_tensor("ab", [1, HID], F32, kind="ExternalInput").ap()
    ag = nc.dram_tensor("ag", [1, HID], F32, kind="ExternalInput").ap()
    abeta = nc.dram_tensor("abeta", [1, HID], F32, kind="ExternalInput").ap()
    abr = nc.dram_tensor("abr", [1, HID], F32, kind="ExternalInput").ap()
    # partition-major layout: [p, ktile, f] so each partition's chunk DMA is
    # one big contiguous run (csz*1956B) instead of csz separate rows
    adjT = {
        ("d", 0): nc.dram_tensor("ad0", [128, NT, R], F8, kind="ExternalInput").ap(),
        ("d", 1): nc.dram_tensor("ad1", [128, NT, R], F8, kind="ExternalInput").ap(),
        ("t", 0): nc.dram_tensor("at0", [128, NT, R], F8, kind="ExternalInput").ap(),
        ("t", 1): nc.dram_tensor("at1", [128, NT, R], F8, kind="ExternalInput").ap(),
    }
    w1cat = nc.dram_tensor("w1cat", [HID, 2 * OUT], BF16, kind="ExternalInput").ap()
    w2in = {
        "d": nc.dram_tensor("w2d", [OUT, OUT], F32, kind="ExternalInput").ap(),
        "t": nc.dram_tensor("w2t", [OUT, OUT], F32, kind="ExternalInput").ap(),
    }
    pa = nc.dram_tensor("pa", [1, 2], F32, kind="ExternalInput").ap()
    zout = {
        "d": nc.dram_tensor("zd", [2, 128, R], F32, kind="ExternalOutput").ap(),
        "t": nc.dram_tensor("zt", [2, 128, R], F32, kind="ExternalOutput").ap(),
    }

    RG = [list(range(N_CORES))]

    with tile.TileContext(nc) as tc:
        import contextlib

        stack = contextlib.ExitStack()
        with stack:
            consts = stack.enter_context(tc.tile_pool(name="consts", bufs=1))
            dram = stack.enter_context(tc.tile_pool(name="dram", bufs=1, space="DRAM"))

            ones_row = consts.tile([1, 128], F32)
            nc.vector.memset(ones_row, 1.0)
            ones_col = consts.tile([128, 1], F32)
            nc.vector.memset(ones_col, 1.0)
            eps1 = consts.tile([1, 1], F32)
            nc.vector.memset(eps1, LN_EPS)
            eps_col = consts.tile([128, 1], F32)
            nc.vector.memset(eps_col, LN_EPS)
            zero_col = consts.tile([128, 1], F32)
            nc.vector.memset(zero_col, 0.0)

            # small weights resident in SBUF
            w2_sb = {}
            for g in ("d", "t"):
                w2t_ = consts.tile([128, 2, OUT], F32, name=f"w2sb_{g}")
                nc.sync.dma_start(
                    out=w2t_, in_=w2in[g].rearrange("(b p) n -> p b n", p=128)
                )
                w2_sb[g] = w2t_
            pa_sb = consts.tile([1, 2], F32)
            nc.sync.dma_start(out=pa_sb, in_=pa)

            # ---------------- Phase A: adapter on this core's segment ----------
            with tc.tile_pool(name="phA", bufs=1) as pA, \
                 tc.tile_pool(name="phA2", bufs=2) as pA2, \
                 tc.tile_pool(name="psA", bufs=2, space="PSUM") as psA:
                identity = pA.tile([128, 128], F32)
                make_identity(nc, identity)

                xseg_sb = pA.tile([128, FEAT // 128, SEG], BF16)
                aW_sb = pA.tile([128, FEAT // 128, HID], BF16)
                aWr_sb = pA.tile([128, FEAT // 128, HID], BF16)
                xsegT_r = xsegT.rearrange("(b p) r -> p b r", p=128)
                aWT_r = aWT.rearrange("(b p) h -> p b h", p=128)
                aWrT_r = aWrT.rearrange("(b p) h -> p b h", p=128)
                for fb in range(FEAT // 128):
                    nc.sync.dma_start(out=xseg_sb[:, fb, :], in_=xsegT_r[:, fb, :])
                    nc.sync.dma_start(out=aW_sb[:, fb, :], in_=aWT_r[:, fb, :])
                    nc.sync.dma_start(out=aWr_sb[:, fb, :], in_=aWrT_r[:, fb, :])
                w1c_sb = pA.tile([128, HID // 128, 2 * OUT], BF16)
                nc.sync.dma_start(
                    out=w1c_sb, in_=w1cat.rearrange("(b p) n -> p b n", p=128)
                )
                ab_sb = pA.tile([1, HID], F32)
                nc.sync.dma_start(out=ab_sb, in_=ab)
                ag_sb = pA.tile([1, HID], F32)
                nc.sync.dma_start(out=ag_sb, in_=ag)
                abeta_sb = pA.tile([1, HID], F32)
                nc.sync.dma_start(out=abeta_sb, in_=abeta)
                abr_sb = pA.tile([1, HID], F32)
                nc.sync.dma_start(out=abr_sb, in_=abr)

                # broadcast g/beta/prelu-a across partitions via rank-1 matmul
                pbc = psA.tile([128, HID], F32, tag="pu")
                nc.tensor.matmul(pbc, ones_row, ag_sb, start=True, stop=True)
                g_rep = pA.tile([128, HID], F32)
                nc.vector.tensor_copy(g_rep, pbc)
                pbc2 = psA.tile([128, HID], F32, tag="pu")
                nc.tensor.matmul(pbc2, ones_row, abeta_sb, start=True, stop=True)
                beta_rep = pA.tile([128, HID], F32)
                nc.vector.tensor_copy(beta_rep, pbc2)
                pbc3 = psA.tile([128, 2], F32, tag="pr")
                nc.tensor.matmul(pbc3, ones_row, pa_sb, start=True, stop=True)
                pa_rep = consts.tile([128, 2], F32)
                nc.vector.tensor_copy(pa_rep, pbc3)

                u_seg = pA.tile([128, SEG // 128, HID], F32)
                r_seg = pA.tile([128, SEG // 128, HID], F32)
                NRT = SEG // 128

                # pass 1: all matmuls (PE dense), PSUM drained to SBUF by DVE
                for rt in range(NRT):
                    pu = psA.tile([128, HID], F32, tag="pu")
                    for fb in range(FEAT // 128):
                        nc.tensor.matmul(
                            pu,
                            xseg_sb[:, fb, rt * 128 : (rt + 1) * 128],
                            aW_sb[:, fb, :],
                            start=(fb == 0),
                            stop=False,
                        )
                    nc.tensor.matmul(pu, ones_row, ab_sb, start=False, stop=True)
                    nc.vector.tensor_copy(u_seg[:, rt, :], pu)
                    pr = psA.tile([128, HID], F32, tag="pr")
                    for fb in range(FEAT // 128):
                        nc.tensor.matmul(
                            pr,
                            xseg_sb[:, fb, rt * 128 : (rt + 1) * 128],
                            aWr_sb[:, fb, :],
                            start=(fb == 0),
                            stop=False,
                        )
                    nc.tensor.matmul(pr, ones_row, abr_sb, start=False, stop=True)
                    nc.vector.tensor_copy(r_seg[:, rt, :], pr)

                # pass 2: LN stats batched; one Sqrt for all tiles
                mv_all = pA2.tile([128, NRT, 2], F32, bufs=1)
                for rt in range(NRT):
                    mv6 = pA2.tile([128, 6], F32)
                    nc.vector.bn_stats(out=mv6, in_=u_seg[:, rt, :])
                    nc.vector.bn_aggr(out=mv_all[:, rt, :], in_=mv6)
                std_all = pA2.tile([128, NRT], F32, bufs=1)
                nc.scalar.activation(
                    out=std_all, in_=mv_all[:, :, 1], func=AF.Sqrt,
                    bias=eps_col, scale=1.0,
                )
                rstd_all = pA2.tile([128, NRT], F32, bufs=1)
                nc.vector.reciprocal(out=rstd_all, in_=std_all)

                # pass 3 (per tile): normalize+affine+relu+residual+elu, then
                # transpose + xW1 projection, split per graph for early gather
                ycontrib = {
                    "d": dram.tile([SEG, OUT], BF16, name="ycontrib_d"),
                    "t": dram.tile([SEG, OUT], BF16, name="ycontrib_t"),
                }
                for rt in range(NRT):
                    t1 = pA2.tile([128, HID], F32)
                    nc.vector.tensor_scalar(
                        out=t1,
                        in0=u_seg[:, rt, :],
                        scalar1=mv_all[:, rt, 0:1],
                        scalar2=rstd_all[:, rt : rt + 1],
                        op0=ALU.subtract,
                        op1=ALU.mult,
                    )
                    nc.gpsimd.tensor_mul(t1, t1, g_rep)
                    nc.gpsimd.tensor_add(t1, t1, beta_rep)
                    nc.vector.tensor_scalar_max(t1, t1, 0.0)
                    nc.vector.tensor_add(t1, t1, r_seg[:, rt, :])
                    # elu(s) = max(s,0) + exp(min(s,0)) - 1
                    smin = pA2.tile([128, HID], F32)
                    nc.gpsimd.tensor_scalar_min(smin, t1, 0.0)
                    ex = pA2.tile([128, HID], F32)
                    nc.scalar.activation(out=ex, in_=smin, func=AF.Exp)
                    nc.vector.tensor_scalar_max(t1, t1, 0.0)
                    nc.vector.tensor_add(t1, t1, ex)
                    nc.vector.tensor_scalar_add(t1, t1, -1.0)
                    # transpose h tile
                    hT_t = pA2.tile([128, HID // 128, 128], BF16)
                    for hb in range(HID // 128):
                        ptr = psA.tile([128, 128], F32, tag="ptr")
                        nc.tensor.transpose(
                            ptr, t1[:, hb * 128 : (hb + 1) * 128], identity
                        )
                        nc.vector.tensor_copy(hT_t[:, hb, :], ptr)
                    # y = h @ [0.5*W1_d | 0.5*W1_t], cast bf16, split per graph
                    py = psA.tile([128, 2 * OUT], F32, tag="pu")
                    for hb in range(HID // 128):
                        nc.tensor.matmul(
                            py,
                            hT_t[:, hb, :],
                            w1c_sb[:, hb, :],
                            start=(hb == 0),
                            stop=(hb == HID // 128 - 1),
                        )
                    ycast = pA2.tile([128, 2 * OUT], BF16)
                    nc.vector.tensor_copy(ycast, py)
                    nc.scalar.dma_start(
                        out=ycontrib["d"][rt * 128 : (rt + 1) * 128, :],
                        in_=ycast[:, 0:OUT],
                    )
                    nc.scalar.dma_start(
                        out=ycontrib["t"][rt * 128 : (rt + 1) * 128, :],
                        in_=ycast[:, OUT : 2 * OUT],
                    )

            # ---------------- gather xW1 across cores (split per graph) ------
            yg = {}
            for g in ("d", "t"):
                ygt = dram.tile(
                    [N_CORES * SEG, OUT], BF16, name=f"yg_{g}", addr_space="Shared"
                )
                nc.gpsimd.collective_compute(
                    "AllGather", ALU.bypass, replica_groups=RG,
                    ins=[ycontrib[g].opt()], outs=[ygt.opt()],
                )
                yg[g] = ygt

            xpool = stack.enter_context(tc.tile_pool(name="xpool", bufs=2))

            def stage_seg(dst, src, seg_idx):
                """copy valid rows of gathered segment into partition-tiled dst.
                Uses the ACT HWDGE ring so it never blocks the adjacency stream."""
                base = seg_idx * SEG
                off = SEG_OFF[seg_idx]
                v = SEG_VALID[seg_idx]
                p0 = off % 128
                hn = (128 - p0) % 128
                cb0 = off // 128
                if hn:
                    nc.scalar.dma_start(
                        out=dst[p0 : p0 + hn, cb0, :],
                        in_=src[base : base + hn, :],
                    )
                cbm = (off + hn) // 128
                nb = (v - hn) // 128
                if nb:
                    nc.scalar.dma_start(
                        out=dst[:, cbm : cbm + nb, :],
                        in_=src[base + hn : base + hn + nb * 128, :].rearrange(
                            "(t p) f -> p t f", p=128
                        ),
                    )
                rem = v - hn - nb * 128
                if rem:
                    nc.scalar.dma_start(
                        out=dst[0:rem, cbm + nb, :],
                        in_=src[base + hn + nb * 128 : base + v, :],
                    )

            def stage_x1(g):
                xt = xpool.tile([128, NT, OUT], BF16, name=f"x1_{g}", tag="X")
                for s_i in range(N_CORES):
                    stage_seg(xt, yg[g], s_i)
                return xt

            # ---------------- Phase B pools ----------------
            apool = stack.enter_context(tc.tile_pool(name="apool", bufs=4))
            hpool = stack.enter_context(tc.tile_pool(name="hpool", bufs=2))
            zpool = stack.enter_context(tc.tile_pool(name="zpool", bufs=2))
            spool = stack.enter_context(tc.tile_pool(name="spool", bufs=1))
            pacc = stack.enter_context(tc.tile_pool(name="pacc", bufs=3, space="PSUM"))
            psm = stack.enter_context(tc.tile_pool(name="psm", bufs=2, space="PSUM"))

            CH = 6
            chunks = []
            k0 = 0
            while k0 < NT:
                chunks.append((k0, min(CH, NT - k0)))
                k0 += CH

            def adj_matmul(x_sb, g, layers):
                """PSUM[mh] <- sum_layers A_layer_shard^T-stream @ X  (transposed out)"""
                acc = [
                    pacc.tile([128, R], F32, name=f"acc{mh}", tag="acc")
                    for mh in range(2)
                ]
                for li, lay in enumerate(layers):
                    a_ap = adjT[(g, lay)]
                    for base_kt, csz in chunks:
                        a_tile = apool.tile([128, CH, R], F8, name="a_tile", tag="a")
                        nc.sync.dma_start(
                            out=a_tile[:, 0:csz, :],
                            in_=a_ap[:, base_kt : base_kt + csz, :],
                        )
                        for j in range(csz):
                            c = base_kt + j
                            first = li == 0 and c == 0
                            last = li == len(layers) - 1 and c == NT - 1
                            kw = 15 if c == NT - 1 else 128
                            for mh in range(2):
                                lhsT = x_sb[0:kw, c, mh * 128 : (mh + 1) * 128]
                                for n0, n1 in ((0, 512), (512, R)):
                                    nc.tensor.matmul(
                                        acc[mh][:, n0:n1],
                                        lhsT,
                                        a_tile[0:kw, j, n0:n1],
                                        start=first,
                                        stop=last,
                                    )
                return acc

            def m1_stage(g, x1):
                """M1 + prelu + hW2 + gather for one graph."""
                acc = adj_matmul(x1, g, (0, 1))
                h_sb = hpool.tile([128, 2, R], F32, name=f"h_{g}", tag="h")
                gi = 0 if g == "d" else 1
                for mh in range(2):
                    # prelu = max(x,0) + a*min(x,0), all on DVE
                    nc.vector.tensor_scalar_max(h_sb[:, mh, :], acc[mh], 0.0)
                    rneg = hpool.tile([128, R], F32, name="rneg", tag="scr978")
                    nc.vector.tensor_scalar(
                        out=rneg,
                        in0=acc[mh],
                        scalar1=zero_col,
                        scalar2=pa_rep[:, gi : gi + 1],
                        op0=ALU.min,
                        op1=ALU.mult,
                    )
                    nc.vector.tensor_add(h_sb[:, mh, :], h_sb[:, mh, :], rneg)
                hcontrib = dram.tile([R, OUT], BF16, name=f"hcontrib_{g}")
                for rt in range(8):
                    rw = min(128, R - rt * 128)
                    ph = psm.tile([128, OUT], F32, name="ph", tag="psm")
                    for nh in range(2):
                        nc.tensor.matmul(
                            ph[0:rw, :],
                            h_sb[:, nh, rt * 128 : rt * 128 + rw],
                            w2_sb[g][:, nh, :],
                            start=(nh == 0),
                            stop=(nh == 1),
                        )
                    hc = hpool.tile([128, OUT], BF16, name="hc", tag="hc")
                    nc.vector.tensor_copy(hc[0:rw, :], ph[0:rw, :])
                    nc.scalar.dma_start(
                        out=hcontrib[rt * 128 : rt * 128 + rw, :], in_=hc[0:rw, :]
                    )
                hgt = dram.tile(
                    [N_CORES * R, OUT], BF16, name=f"hg_{g}", addr_space="Shared"
                )
                nc.gpsimd.collective_compute(
                    "AllGather", ALU.bypass, replica_groups=RG,
                    ins=[hcontrib.opt()], outs=[hgt.opt()],
                )
                return hgt

            x1_d = stage_x1("d")
            hg_d = m1_stage("d", x1_d)
            x1_t = stage_x1("t")
            hg_t = m1_stage("t", x1_t)
            hg = {"d": hg_d, "t": hg_t}

            # ---------------- M2 per graph + stats collective ----------------
            zs = {}
            stats = {}
            for g in ("d", "t"):
                # stage X2 = gathered hW2 (rows 0..7823 + zero pad)
                x2 = xpool.tile([128, NT, OUT], BF16, name=f"x2_{g}", tag="X")
                nc.scalar.dma_start(
                    out=x2[:, 0 : NT - 1, :],
                    in_=hg[g][0 : (NT - 1) * 128, :].rearrange(
                        "(t p) f -> p t f", p=128
                    ),
                )
                nc.scalar.dma_start(
                    out=x2[0:16, NT - 1, :],
                    in_=hg[g][(NT - 1) * 128 : (NT - 1) * 128 + 16, :],
                )

                acc = adj_matmul(x2, g, (0, 1))
                z_sb = zpool.tile([128, 2, R], F32, name=f"z_{g}", tag="z")
                zsq = hpool.tile([128, R], F32, name="zsq", tag="scr978")
                s_sb = spool.tile([1, R], F32, name=f"s_{g}", tag=f"s_{g}")
                q_sb = spool.tile([1, R], F32, name=f"q_{g}", tag=f"q_{g}")
                # column sums of Z' and Z'^2 (per-node-row partial norms)
                for mh in range(2):
                    nc.vector.tensor_copy(z_sb[:, mh, :], acc[mh])
                for n0, n1 in ((0, 512), (512, R)):
                    pred = psm.tile([1, 512], F32, name="pred", tag="psm")
                    for mh in range(2):
                        nc.tensor.matmul(
                            pred[:, 0 : n1 - n0],
                            ones_col,
                            z_sb[:, mh, n0:n1],
                            start=(mh == 0),
                            stop=(mh == 1),
                        )
                    nc.vector.tensor_copy(s_sb[:, n0:n1], pred[:, 0 : n1 - n0])
                for mh in range(2):
                    nc.vector.tensor_mul(zsq, z_sb[:, mh, :], z_sb[:, mh, :])
                    for n0, n1 in ((0, 512), (512, R)):
                        pred2 = psm.tile([1, 512], F32, name="pred2", tag="psm")
                        nc.tensor.matmul(
                            pred2[:, 0 : n1 - n0],
                            ones_col,
                            zsq[:, n0:n1],
                            start=True,
                            stop=True,
                        )
                        if mh == 0:
                            nc.vector.tensor_copy(
                                q_sb[:, n0:n1], pred2[:, 0 : n1 - n0]
                            )
                        else:
                            nc.vector.tensor_add(
                                q_sb[:, n0:n1], q_sb[:, n0:n1], pred2[:, 0 : n1 - n0]
                            )

                # local S, SS then AllReduce (gpsimd DMAs keep HWDGE rings free)
                ssum = spool.tile([1, 2], F32, name=f"ssum_{g}", tag=f"ssum_{g}")
                nc.vector.tensor_reduce(
                    ssum[:, 0:1], s_sb, axis=mybir.AxisListType.X, op=ALU.add
                )
                nc.vector.tensor_reduce(
                    ssum[:, 1:2], q_sb, axis=mybir.AxisListType.X, op=ALU.add
                )
                statc = dram.tile([1, 2], F32, name=f"statc_{g}")
                nc.gpsimd.dma_start(out=statc, in_=ssum)
                statg = dram.tile([1, 2], F32, name=f"statg_{g}")
                nc.gpsimd.collective_compute(
                    "AllReduce", ALU.add, replica_groups=RG,
                    ins=[statc.opt()], outs=[statg.opt()],
                )
                sg = spool.tile([1, 2], F32, name=f"sg_{g}", tag=f"sg_{g}")
                nc.gpsimd.dma_start(out=sg, in_=statg)
                zs[g] = z_sb
                stats[g] = (s_sb, q_sb, sg)

            # ---------------- tails: LN + row-norm + output ----------------
            for g in ("d", "t"):
                z_sb = zs[g]
                s_sb, q_sb, sg = stats[g]
                mu = spool.tile([1, 2], F32, name="mu", tag="mu")
                nc.vector.tensor_scalar_mul(mu, sg, 1.0 / NTOT)  # (m, E[z^2])
                msq = spool.tile([1, 1], F32, name="msq", tag="msq")
                nc.vector.tensor_mul(msq, mu[:, 0:1], mu[:, 0:1])
                var = spool.tile([1, 1], F32, name="var", tag="var")
                nc.vector.tensor_sub(var, mu[:, 1:2], msq)
                stdg = spool.tile([1, 1], F32, name="stdg", tag="stdg")
                nc.scalar.activation(
                    out=stdg, in_=var, func=AF.Sqrt, bias=eps1, scale=1.0
                )
                rstdg = spool.tile([1, 1], F32, name="rstdg", tag="rstdg")
                nc.vector.reciprocal(out=rstdg, in_=stdg)

                # rstd*||z-m||_r = sqrt((q - 2m*s)*rstd^2 + 256*m^2*rstd^2)
                t2m = spool.tile([1, 1], F32, name="t2m", tag="t2m")
                nc.vector.tensor_scalar_mul(t2m, mu[:, 0:1], 2.0)
                rstd2 = spool.tile([1, 1], F32, name="rstd2", tag="rstd2")
                nc.vector.tensor_mul(rstd2, rstdg, rstdg)
                bias2 = spool.tile([1, 1], F32, name="bias2", tag="bias2")
                nc.vector.tensor_mul(bias2, msq, rstd2)
                nc.vector.tensor_scalar_mul(bias2, bias2, float(OUT))
                cvec = spool.tile([1, R], F32, name="cvec", tag="cvec")
                nc.vector.tensor_scalar(
                    out=cvec, in0=s_sb, scalar1=t2m, scalar2=None, op0=ALU.mult
                )
                nc.vector.tensor_sub(cvec, q_sb, cvec)
                nc.scalar.activation(
                    out=cvec, in_=cvec, func=AF.Sqrt, bias=bias2, scale=rstd2
                )
                nc.vector.tensor_scalar_add(cvec, cvec, 1e-8)
                nc.vector.reciprocal(out=cvec, in_=cvec)
                nc.vector.tensor_scalar(
                    out=cvec, in0=cvec, scalar1=rstdg, scalar2=None, op0=ALU.mult
                )
                # broadcast row-scale and global mean across partitions
                sb_s = spool.tile([128, R], F32, name="sb_s", tag="sb_s")
                for n0, n1 in ((0, 512), (512, R)):
                    pb = psm.tile([128, 512], F32, name="pb", tag="psm")
                    nc.tensor.matmul(
                        pb[:, 0 : n1 - n0], ones_row, cvec[:, n0:n1],
                        start=True, stop=True,
                    )
                    nc.vector.tensor_copy(sb_s[:, n0:n1], pb[:, 0 : n1 - n0])
                pmc = psm.tile([128, 1], F32, name="pmc", tag="psm")
                nc.tensor.matmul(pmc, ones_row, mu[:, 0:1], start=True, stop=True)
                m_col = spool.tile([128, 1], F32, name="m_col", tag="m_col")
                nc.vector.tensor_copy(m_col, pmc)
                # z_final = (z - m) * s_b ; write out
                for mh in range(2):
                    nc.vector.tensor_scalar(
                        out=z_sb[:, mh, :], in0=z_sb[:, mh, :],
                        scalar1=m_col, scalar2=None, op0=ALU.subtract,
                    )
                    nc.vector.tensor_mul(z_sb[:, mh, :], z_sb[:, mh, :], sb_s)
                    nc.scalar.dma_start(out=zout[g][mh, :, :], in_=z_sb[:, mh, :])

    nc.compile()
    return nc


def _get_program():
    if _PROGRAM["nc"] is None:
        _PROGRAM["nc"] = _build_program()
    return _PROGRAM["nc"]


def _shard_inputs(inputs):
    f32 = np.float32
    bf16 = ml_dtypes.bfloat16
    x_all = np.concatenate(
        [
            np.asarray(inputs["drug_feats"], f32),
            np.asarray(inputs["target_feats"], f32),
            np.asarray(inputs["disease_feats"], f32),
        ],
        axis=0,
    )
    adjs = {
        "d": np.asarray(inputs["drug_adjs"], f32),
        "t": np.asarray(inputs["target_adjs"], f32),
    }
    # transposed row-shards in fp8e4m3 (A-side quantization noise averages
    # out; X stays bf16), zero-padded, partition-major tiling [p, ktile, f]
    f8np = mybir.dt.np(F8)
    adjT_shards = {}
    for g in ("d", "t"):
        for lay in (0, 1):
            at = np.ascontiguousarray(adjs[g][lay].T)  # [N, N] col r-major
            per_core = []
            for c in range(N_CORES):
                r0 = c * R
                r1 = min(r0 + R, N_NODES)
                sh = np.zeros((PAD_N, R), dtype=f8np)
                sh[:N_NODES, : r1 - r0] = at[:, r0:r1].astype(f8np)
                sh = np.ascontiguousarray(
                    sh.reshape(NT, 128, R).transpose(1, 0, 2)
                )  # [128, NT, R]
                per_core.append(sh)
            adjT_shards[(g, lay)] = per_core

    aw = {
        0: ("dA", inputs),
        1: ("tA", inputs),
        2: ("sA", inputs),
    }

    w1cat = (
        np.concatenate(
            [np.asarray(inputs["dG_W1"], f32), np.asarray(inputs["tG_W1"], f32)],
            axis=1,
        )
        * np.float32(0.5)
    ).astype(bf16)
    w2d = np.asarray(inputs["dG_W2"], f32) * np.float32(0.5)
    w2t = np.asarray(inputs["tG_W2"], f32) * np.float32(0.5)
    pa_arr = np.array(
        [[np.asarray(inputs["dG_a"], f32)[0], np.asarray(inputs["tG_a"], f32)[0]]], f32
    )

    in_maps = []
    for c in range(N_CORES):
        ty = SEG_TYPE[c]
        pref = aw[ty][0]
        off, v = SEG_OFF[c], SEG_VALID[c]
        xsegT = np.zeros((FEAT, SEG), bf16)
        xsegT[:, :v] = x_all[off : off + v].T.astype(bf16)
        m = {
            "xsegT": xsegT,
            "aWT": np.ascontiguousarray(np.asarray(inputs[pref + "_W"], f32).T).astype(bf16),
            "aWrT": np.ascontiguousarray(np.asarray(inputs[pref + "_Wr"], f32).T).astype(bf16),
            "ab": np.asarray(inputs[pref + "_b"], f32).reshape(1, HID),
            "ag": np.asarray(inputs[pref + "_g"], f32).reshape(1, HID),
            "abeta": np.asarray(inputs[pref + "_beta"], f32).reshape(1, HID),
            "abr": np.asarray(inputs[pref + "_br"], f32).reshape(1, HID),
            "ad0": adjT_shards[("d", 0)][c],
            "ad1": adjT_shards[("d", 1)][c],
            "at0": adjT_shards[("t", 0)][c],
            "at1": adjT_shards[("t", 1)][c],
            "w1cat": w1cat,
            "w2d": w2d,
            "w2t": w2t,
            "pa": pa_arr,
        }
        in_maps.append(m)
    return in_maps


def run_on_device(inputs, trace=False):
    nc = _get_program()
    in_maps = _shard_inputs(inputs)
    res = bass_utils.run_bass_kernel_spmd(
        nc, in_maps, core_ids=list(range(N_CORES)), trace=trace
    )
    # assemble
    def shard_rows(core, name):
        return (
            np.asarray(res.results[core][name]).reshape(2 * 128, R).T.astype(np.float32)
        )

    zd0 = shard_rows(0, "zd")
    z_drug = np.ascontiguousarray(zd0[:N_DRUG])
    zt0 = shard_rows(0, "zt")
    zt1 = shard_rows(1, "zt")
    zt2 = shard_rows(2, "zt")
    z_target = np.ascontiguousarray(
        np.concatenate([zt0[N_DRUG:R], zt1, zt2[: N_DRUG + N_TARGET - 2 * R]], axis=0)
    )
    return (z_drug, z_target), res


def kernel(**inputs):
    out, _ = run_on_device(inputs, trace=False)
    return out


# revision 34
# speedup vs baseline: 1.1096x; 1.1096x over previous
"""Trainium2 Bass kernel for nn_CoarseView (gnn_message_passing).

Self-contained: host shards FULL inputs across 8 NeuronCores, runs one
SPMD Bass/Tile program (adapters -> gather xW1 -> adj matmuls with
collective gathers between stages -> global LN + row-norm), gathers the
FULL output.

Sharding: 1D row partition of the adjacency matmuls (978 rows/core,
core 7 zero-padded). Adjacency row-shards are transposed on the host so
the contraction index lands on SBUF partitions with contiguous DMAs, and
cast to bf16 to halve the ~2 GB HBM stream. Adapter/GCN weights are
replicated; the 2-layer adjacency mean is folded into 0.5*W1 / 0.5*W2.
"""

import sys
import types

import numpy as np
import ml_dtypes

# ---- NTFF profile hook shim (this image's antenv lacks axon_hooks; the
# boot-time registration degrades silently without it). Harmless if unused.
def _install_axon_hook_shim():
    try:
        import antenv  # noqa: F401
    except Exception:
        return
    if "antenv.axon_hooks" in sys.modules:
        return
    mod = types.ModuleType("antenv.axon_hooks")
    _h = {"hook": None}
    mod.set_axon_ntff_profile_hook = lambda h: _h.__setitem__("hook", h)
    mod.get_axon_ntff_profile_hook = lambda: _h["hook"]
    sys.modules["antenv.axon_hooks"] = mod
    try:
        import antenv as _a

        _a.axon_hooks = mod
        from trn_agent_boot.trn_boot import _ntff_profile_via_ctypes

        hook = _ntff_profile_via_ctypes("/opt/axon/libaxon_pjrt.so")
        mod.set_axon_ntff_profile_hook(hook)
    except Exception:
        pass


_install_axon_hook_shim()

import concourse.bacc as bacc
import concourse.tile as tile
from concourse import mybir
from concourse import bass_utils
from concourse.masks import make_identity

F32 = mybir.dt.float32
BF16 = mybir.dt.bfloat16
F8 = mybir.dt.float8e4
AF = mybir.ActivationFunctionType
ALU = mybir.AluOpType

N_CORES = 8
N_NODES = 7823
N_DRUG = 708
N_TARGET = 1512
FEAT = 1024
HID = 512
OUT = 256
LN_EPS = 1e-5

R = 978  # adjacency rows per core (core 7: 977 real + 1 zero pad)
PAD_N = 7936  # 62 * 128, padded contraction length
NT = PAD_N // 128  # 62 K-tiles
SEG = 1152  # 9 * 128, padded adapter segment rows
SEG_OFF = [0, 708, 1464, 2220, 3341, 4462, 5583, 6704]
SEG_VALID = [708, 756, 756, 1121, 1121, 1121, 1121, 1119]
SEG_TYPE = [0, 1, 1, 2, 2, 2, 2, 2]  # 0=drug(dA) 1=target(tA) 2=disease(sA)
NTOT = N_NODES * OUT  # global-LN element count

_PROGRAM = {"nc": None}


def _build_program():
    nc = bacc.Bacc(
        "TRN2",
        target_bir_lowering=False,
        debug=False,
        enable_asserts=False,
        num_devices=N_CORES,
    )

    # ---- I/O ----
    xsegT = nc.dram_tensor("xsegT", [FEAT, SEG], BF16, kind="ExternalInput").ap()
    aWT = nc.dram_tensor("aWT", [FEAT, HID], BF16, kind="ExternalInput").ap()
    aWrT = nc.dram_tensor("aWrT", [FEAT, HID], BF16, kind="ExternalInput").ap()
    ab = nc.dram_tensor("ab", [1, HID], F32, kind="ExternalInput").ap()
    ag = nc.dram_tensor("ag", [1, HID], F32, kind="ExternalInput").ap()
    abeta = nc.dram_tensor("abeta", [1, HID], F32, kind="ExternalInput").ap()
    abr = nc.dram_tensor("abr", [1, HID], F32, kind="ExternalInput").ap()
    # partition-major layout: [p, ktile, f] so each partition's chunk DMA is
    # one big contiguous run (csz*1956B) instead of csz separate rows
    adjT = {
        ("d", 0): nc.dram_tensor("ad0", [128, NT, R], F8, kind="ExternalInput").ap(),
        ("d", 1): nc.dram_tensor("ad1", [128, NT, R], F8, kind="ExternalInput").ap(),
        ("t", 0): nc.dram_tensor("at0", [128, NT, R], F8, kind="ExternalInput").ap(),
        ("t", 1): nc.dram_tensor("at1", [128, NT, R], F8, kind="ExternalInput").ap(),
    }
    w1cat = nc.dram_tensor("w1cat", [HID, 2 * OUT], BF16, kind="ExternalInput").ap()
    w2in = {
        "d": nc.dram_tensor("w2d", [OUT, OUT], F32, kind="ExternalInput").ap(),
        "t": nc.dram_tensor("w2t", [OUT, OUT], F32, kind="ExternalInput").ap(),
    }
    pa = nc.dram_tensor("pa", [1, 2], F32, kind="ExternalInput").ap()
    zout = {
        "d": nc.dram_tensor("zd", [2, 128, R], F32, kind="ExternalOutput").ap(),
        "t": nc.dram_tensor("zt", [2, 128, R], F32, kind="ExternalOutput").ap(),
    }

    RG = [list(range(N_CORES))]

    with tile.TileContext(nc) as tc:
        import contextlib

        stack = contextlib.ExitStack()
        with stack:
            consts = stack.enter_context(tc.tile_pool(name="consts", bufs=1))
            dram = stack.enter_context(tc.tile_pool(name="dram", bufs=1, space="DRAM"))

            ones_row = consts.tile([1, 128], F32)
            nc.vector.memset(ones_row, 1.0)
            ones_col = consts.tile([128, 1], F32)
            nc.vector.memset(ones_col, 1.0)
            eps1 = consts.tile([1, 1], F32)
            nc.vector.memset(eps1, LN_EPS)
            eps_col = consts.tile([128, 1], F32)
            nc.vector.memset(eps_col, LN_EPS)
            zero_col = consts.tile([128, 1], F32)
            nc.vector.memset(zero_col, 0.0)

            # small weights resident in SBUF
            w2_sb = {}
            for g in ("d", "t"):
                w2t_ = consts.tile([128, 2, OUT], F32, name=f"w2sb_{g}")
                nc.sync.dma_start(
                    out=w2t_, in_=w2in[g].rearrange("(b p) n -> p b n", p=128)
                )
                w2_sb[g] = w2t_
            pa_sb = consts.tile([1, 2], F32)
            nc.sync.dma_start(out=pa_sb, in_=pa)

            # ---------------- Phase A: adapter on this core's segment ----------
            with tc.tile_pool(name="phA", bufs=1) as pA, \
                 tc.tile_pool(name="phA2", bufs=2) as pA2, \
                 tc.tile_pool(name="psA", bufs=2, space="PSUM") as psA:
                identity = pA.tile([128, 128], F32)
                make_identity(nc, identity)

                xseg_sb = pA.tile([128, FEAT // 128, SEG], BF16)
                aW_sb = pA.tile([128, FEAT // 128, HID], BF16)
                aWr_sb = pA.tile([128, FEAT // 128, HID], BF16)
                xsegT_r = xsegT.rearrange("(b p) r -> p b r", p=128)
                aWT_r = aWT.rearrange("(b p) h -> p b h", p=128)
                aWrT_r = aWrT.rearrange("(b p) h -> p b h", p=128)
                for fb in range(FEAT // 128):
                    nc.sync.dma_start(out=xseg_sb[:, fb, :], in_=xsegT_r[:, fb, :])
                    nc.sync.dma_start(out=aW_sb[:, fb, :], in_=aWT_r[:, fb, :])
                    nc.sync.dma_start(out=aWr_sb[:, fb, :], in_=aWrT_r[:, fb, :])
                w1c_sb = pA.tile([128, HID // 128, 2 * OUT], BF16)
                nc.sync.dma_start(
                    out=w1c_sb, in_=w1cat.rearrange("(b p) n -> p b n", p=128)
                )
                ab_sb = pA.tile([1, HID], F32)
                nc.sync.dma_start(out=ab_sb, in_=ab)
                ag_sb = pA.tile([1, HID], F32)
                nc.sync.dma_start(out=ag_sb, in_=ag)
                abeta_sb = pA.tile([1, HID], F32)
                nc.sync.dma_start(out=abeta_sb, in_=abeta)
                abr_sb = pA.tile([1, HID], F32)
                nc.sync.dma_start(out=abr_sb, in_=abr)

                # broadcast g/beta/prelu-a across partitions via rank-1 matmul
                pbc = psA.tile([128, HID], F32, tag="pu")
                nc.tensor.matmul(pbc, ones_row, ag_sb, start=True, stop=True)
                g_rep = pA.tile([128, HID], F32)
                nc.vector.tensor_copy(g_rep, pbc)
                pbc2 = psA.tile([128, HID], F32, tag="pu")
                nc.tensor.matmul(pbc2, ones_row, abeta_sb, start=True, stop=True)
                beta_rep = pA.tile([128, HID], F32)
                nc.vector.tensor_copy(beta_rep, pbc2)
                pbc3 = psA.tile([128, 2], F32, tag="pr")
                nc.tensor.matmul(pbc3, ones_row, pa_sb, start=True, stop=True)
                pa_rep = consts.tile([128, 2], F32)
                nc.vector.tensor_copy(pa_rep, pbc3)

                u_seg = pA.tile([128, SEG // 128, HID], F32)
                r_seg = pA.tile([128, SEG // 128, HID], F32)
                NRT = SEG // 128

                # pass 1: all matmuls (PE dense), PSUM drained to SBUF by DVE
                for rt in range(NRT):
                    pu = psA.tile([128, HID], F32, tag="pu")
                    for fb in range(FEAT // 128):
                        nc.tensor.matmul(
                            pu,
                            xseg_sb[:, fb, rt * 128 : (rt + 1) * 128],
                            aW_sb[:, fb, :],
                            start=(fb == 0),
                            stop=False,
                        )
                    nc.tensor.matmul(pu, ones_row, ab_sb, start=False, stop=True)
                    nc.vector.tensor_copy(u_seg[:, rt, :], pu)
                    pr = psA.tile([128, HID], F32, tag="pr")
                    for fb in range(FEAT // 128):
                        nc.tensor.matmul(
                            pr,
                            xseg_sb[:, fb, rt * 128 : (rt + 1) * 128],
                            aWr_sb[:, fb, :],
                            start=(fb == 0),
                            stop=False,
                        )
                    nc.tensor.matmul(pr, ones_row, abr_sb, start=False, stop=True)
                    nc.vector.tensor_copy(r_seg[:, rt, :], pr)

                # pass 2: LN stats batched; one Sqrt for all tiles
                mv_all = pA2.tile([128, NRT, 2], F32, bufs=1)
                for rt in range(NRT):
                    mv6 = pA2.tile([128, 6], F32)
                    nc.vector.bn_stats(out=mv6, in_=u_seg[:, rt, :])
                    nc.vector.bn_aggr(out=mv_all[:, rt, :], in_=mv6)
                std_all = pA2.tile([128, NRT], F32, bufs=1)
                nc.scalar.activation(
                    out=std_all, in_=mv_all[:, :, 1], func=AF.Sqrt,
                    bias=eps_col, scale=1.0,
                )
                rstd_all = pA2.tile([128, NRT], F32, bufs=1)
                nc.vector.reciprocal(out=rstd_all, in_=std_all)

                # pass 3 (per tile): normalize+affine+relu+residual+elu, then
                # transpose + xW1 projection, split per graph for early gather
                ycontrib = {
                    "d": dram.tile([SEG, OUT], BF16, name="ycontrib_d"),
                    "t": dram.tile([SEG, OUT], BF16, name="ycontrib_t"),
                }
                for rt in range(NRT):
                    t1 = pA2.tile([128, HID], F32)
                    nc.vector.tensor_scalar(
                        out=t1,
                        in0=u_seg[:, rt, :],
                        scalar1=mv_all[:, rt, 0:1],
                        scalar2=rstd_all[:, rt : rt + 1],
                        op0=ALU.subtract,
                        op1=ALU.mult,
                    )
                    nc.vector.tensor_mul(t1, t1, g_rep)
                    nc.vector.tensor_add(t1, t1, beta_rep)
                    nc.vector.tensor_scalar_max(t1, t1, 0.0)
                    nc.vector.tensor_add(t1, t1, r_seg[:, rt, :])
                    # elu(s) = max(s,0) + exp(min(s,0)) - 1
                    smin = pA2.tile([128, HID], F32)
                    nc.vector.tensor_scalar_min(smin, t1, 0.0)
                    ex = pA2.tile([128, HID], F32)
                    nc.scalar.activation(out=ex, in_=smin, func=AF.Exp)
                    nc.vector.tensor_scalar_max(t1, t1, 0.0)
                    nc.vector.tensor_add(t1, t1, ex)
                    nc.vector.tensor_scalar_add(t1, t1, -1.0)
                    # transpose h tile
                    hT_t = pA2.tile([128, HID // 128, 128], BF16)
                    for hb in range(HID // 128):
                        ptr = psA.tile([128, 128], F32, tag="ptr")
                        nc.tensor.transpose(
                            ptr, t1[:, hb * 128 : (hb + 1) * 128], identity
                        )
                        nc.vector.tensor_copy(hT_t[:, hb, :], ptr)
                    # y = h @ [0.5*W1_d | 0.5*W1_t], cast bf16, split per graph
                    py = psA.tile([128, 2 * OUT], F32, tag="pu")
                    for hb in range(HID // 128):
                        nc.tensor.matmul(
                            py,
                            hT_t[:, hb, :],
                            w1c_sb[:, hb, :],
                            start=(hb == 0),
                            stop=(hb == HID // 128 - 1),
                        )
                    ycast = pA2.tile([128, 2 * OUT], BF16)
                    nc.vector.tensor_copy(ycast, py)
                    nc.scalar.dma_start(
                        out=ycontrib["d"][rt * 128 : (rt + 1) * 128, :],
                        in_=ycast[:, 0:OUT],
                    )
                    nc.scalar.dma_start(
                        out=ycontrib["t"][rt * 128 : (rt + 1) * 128, :],
                        in_=ycast[:, OUT : 2 * OUT],
                    )

            # ---------------- gather xW1 across cores (split per graph) ------
            yg = {}
            for g in ("d", "t"):
                ygt = dram.tile(
                    [N_CORES * SEG, OUT], BF16, name=f"yg_{g}", addr_space="Shared"
                )
                nc.gpsimd.collective_compute(
                    "AllGather", ALU.bypass, replica_groups=RG,
                    ins=[ycontrib[g].opt()], outs=[ygt.opt()],
                )
                yg[g] = ygt

            xpool = stack.enter_context(tc.tile_pool(name="xpool", bufs=2))

            def stage_seg(dst, src, seg_idx):
                """copy valid rows of gathered segment into partition-tiled dst.
                Uses the ACT HWDGE ring so it never blocks the adjacency stream."""
                base = seg_idx * SEG
                off = SEG_OFF[seg_idx]
                v = SEG_VALID[seg_idx]
                p0 = off % 128
                hn = (128 - p0) % 128
                cb0 = off // 128
                if hn:
                    nc.scalar.dma_start(
                        out=dst[p0 : p0 + hn, cb0, :],
                        in_=src[base : base + hn, :],
                    )
                cbm = (off + hn) // 128
                nb = (v - hn) // 128
                if nb:
                    nc.scalar.dma_start(
                        out=dst[:, cbm : cbm + nb, :],
                        in_=src[base + hn : base + hn + nb * 128, :].rearrange(
                            "(t p) f -> p t f", p=128
                        ),
                    )
                rem = v - hn - nb * 128
                if rem:
                    nc.scalar.dma_start(
                        out=dst[0:rem, cbm + nb, :],
                        in_=src[base + hn + nb * 128 : base + v, :],
                    )

            def stage_x1(g):
                xt = xpool.tile([128, NT, OUT], BF16, name=f"x1_{g}", tag="X")
                for s_i in range(N_CORES):
                    stage_seg(xt, yg[g], s_i)
                return xt

            # ---------------- Phase B pools ----------------
            apool = stack.enter_context(tc.tile_pool(name="apool", bufs=4))
            hpool = stack.enter_context(tc.tile_pool(name="hpool", bufs=2))
            zpool = stack.enter_context(tc.tile_pool(name="zpool", bufs=2))
            spool = stack.enter_context(tc.tile_pool(name="spool", bufs=1))
            pacc = stack.enter_context(tc.tile_pool(name="pacc", bufs=3, space="PSUM"))
            psm = stack.enter_context(tc.tile_pool(name="psm", bufs=2, space="PSUM"))

            CH = 6
            chunks = []
            k0 = 0
            while k0 < NT:
                chunks.append((k0, min(CH, NT - k0)))
                k0 += CH

            def adj_matmul(x_sb, g, layers):
                """PSUM[mh] <- sum_layers A_layer_shard^T-stream @ X  (transposed out)"""
                acc = [
                    pacc.tile([128, R], F32, name=f"acc{mh}", tag="acc")
                    for mh in range(2)
                ]
                for li, lay in enumerate(layers):
                    a_ap = adjT[(g, lay)]
                    for base_kt, csz in chunks:
                        a_tile = apool.tile([128, CH, R], F8, name="a_tile", tag="a")
                        nc.sync.dma_start(
                            out=a_tile[:, 0:csz, :],
                            in_=a_ap[:, base_kt : base_kt + csz, :],
                        )
                        for j in range(csz):
                            c = base_kt + j
                            first = li == 0 and c == 0
                            last = li == len(layers) - 1 and c == NT - 1
                            kw = 15 if c == NT - 1 else 128
                            for mh in range(2):
                                lhsT = x_sb[0:kw, c, mh * 128 : (mh + 1) * 128]
                                for n0, n1 in ((0, 512), (512, R)):
                                    nc.tensor.matmul(
                                        acc[mh][:, n0:n1],
                                        lhsT,
                                        a_tile[0:kw, j, n0:n1],
                                        start=first,
                                        stop=last,
                                    )
                return acc

            def m1_stage(g, x1):
                """M1 + prelu + hW2 + gather for one graph."""
                acc = adj_matmul(x1, g, (0, 1))
                h_sb = hpool.tile([128, 2, R], F32, name=f"h_{g}", tag="h")
                gi = 0 if g == "d" else 1
                for mh in range(2):
                    # prelu = max(x,0) + a*min(x,0), all on DVE
                    nc.vector.tensor_scalar_max(h_sb[:, mh, :], acc[mh], 0.0)
                    rneg = hpool.tile([128, R], F32, name="rneg", tag="scr978")
                    nc.vector.tensor_scalar(
                        out=rneg,
                        in0=acc[mh],
                        scalar1=zero_col,
                        scalar2=pa_rep[:, gi : gi + 1],
                        op0=ALU.min,
                        op1=ALU.mult,
                    )
                    nc.vector.tensor_add(h_sb[:, mh, :], h_sb[:, mh, :], rneg)
                hcontrib = dram.tile([R, OUT], BF16, name=f"hcontrib_{g}")
                for rt in range(8):
                    rw = min(128, R - rt * 128)
                    ph = psm.tile([128, OUT], F32, name="ph", tag="psm")
                    for nh in range(2):
                        nc.tensor.matmul(
                            ph[0:rw, :],
                            h_sb[:, nh, rt * 128 : rt * 128 + rw],
                            w2_sb[g][:, nh, :],
                            start=(nh == 0),
                            stop=(nh == 1),
                        )
                    hc = hpool.tile([128, OUT], BF16, name="hc", tag="hc")
                    nc.vector.tensor_copy(hc[0:rw, :], ph[0:rw, :])
                    nc.scalar.dma_start(
                        out=hcontrib[rt * 128 : rt * 128 + rw, :], in_=hc[0:rw, :]
                    )
                hgt = dram.tile(
                    [N_CORES * R, OUT], BF16, name=f"hg_{g}", addr_space="Shared"
                )
                nc.gpsimd.collective_compute(
                    "AllGather", ALU.bypass, replica_groups=RG,
                    ins=[hcontrib.opt()], outs=[hgt.opt()],
                )
                return hgt

            x1_d = stage_x1("d")
            hg_d = m1_stage("d", x1_d)
            x1_t = stage_x1("t")
            hg_t = m1_stage("t", x1_t)
            hg = {"d": hg_d, "t": hg_t}

            # ---------------- M2 per graph + stats collective ----------------
            zs = {}
            stats = {}
            for g in ("d", "t"):
                # stage X2 = gathered hW2 (rows 0..7823 + zero pad)
                x2 = xpool.tile([128, NT, OUT], BF16, name=f"x2_{g}", tag="X")
                nc.scalar.dma_start(
                    out=x2[:, 0 : NT - 1, :],
                    in_=hg[g][0 : (NT - 1) * 128, :].rearrange(
                        "(t p) f -> p t f", p=128
                    ),
                )
                nc.scalar.dma_start(
                    out=x2[0:16, NT - 1, :],
                    in_=hg[g][(NT - 1) * 128 : (NT - 1) * 128 + 16, :],
                )

                acc = adj_matmul(x2, g, (0, 1))
                z_sb = zpool.tile([128, 2, R], F32, name=f"z_{g}", tag="z")
                zsq = hpool.tile([128, R], F32, name="zsq", tag="scr978")
                s_sb = spool.tile([1, R], F32, name=f"s_{g}", tag=f"s_{g}")
                q_sb = spool.tile([1, R], F32, name=f"q_{g}", tag=f"q_{g}")
                # column sums of Z' and Z'^2 (per-node-row partial norms)
                for mh in range(2):
                    nc.vector.tensor_copy(z_sb[:, mh, :], acc[mh])
                for n0, n1 in ((0, 512), (512, R)):
                    pred = psm.tile([1, 512], F32, name="pred", tag="psm")
                    for mh in range(2):
                        nc.tensor.matmul(
                            pred[:, 0 : n1 - n0],
                            ones_col,
                            z_sb[:, mh, n0:n1],
                            start=(mh == 0),
                            stop=(mh == 1),
                        )
                    nc.vector.tensor_copy(s_sb[:, n0:n1], pred[:, 0 : n1 - n0])
                for mh in range(2):
                    nc.vector.tensor_mul(zsq, z_sb[:, mh, :], z_sb[:, mh, :])
                    for n0, n1 in ((0, 512), (512, R)):
                        pred2 = psm.tile([1, 512], F32, name="pred2", tag="psm")
                        nc.tensor.matmul(
                            pred2[:, 0 : n1 - n0],
                            ones_col,
                            zsq[:, n0:n1],
                            start=True,
                            stop=True,
                        )
                        if mh == 0:
                            nc.vector.tensor_copy(
                                q_sb[:, n0:n1], pred2[:, 0 : n1 - n0]
                            )
                        else:
                            nc.vector.tensor_add(
                                q_sb[:, n0:n1], q_sb[:, n0:n1], pred2[:, 0 : n1 - n0]
                            )

                # local S, SS then AllReduce (gpsimd DMAs keep HWDGE rings free)
                ssum = spool.tile([1, 2], F32, name=f"ssum_{g}", tag=f"ssum_{g}")
                nc.vector.tensor_reduce(
                    ssum[:, 0:1], s_sb, axis=mybir.AxisListType.X, op=ALU.add
                )
                nc.vector.tensor_reduce(
                    ssum[:, 1:2], q_sb, axis=mybir.AxisListType.X, op=ALU.add
                )
                statc = dram.tile([1, 2], F32, name=f"statc_{g}")
                nc.gpsimd.dma_start(out=statc, in_=ssum)
                statg = dram.tile([1, 2], F32, name=f"statg_{g}")
                nc.gpsimd.collective_compute(
                    "AllReduce", ALU.add, replica_groups=RG,
                    ins=[statc.opt()], outs=[statg.opt()],
                )
                sg = spool.tile([1, 2], F32, name=f"sg_{g}", tag=f"sg_{g}")
                nc.gpsimd.dma_start(out=sg, in_=statg)
                zs[g] = z_sb
                stats[g] = (s_sb, q_sb, sg)

            # ---------------- tails: LN + row-norm + output ----------------
            for g in ("d", "t"):
                z_sb = zs[g]
                s_sb, q_sb, sg = stats[g]
                mu = spool.tile([1, 2], F32, name="mu", tag="mu")
                nc.vector.tensor_scalar_mul(mu, sg, 1.0 / NTOT)  # (m, E[z^2])
                msq = spool.tile([1, 1], F32, name="msq", tag="msq")
                nc.vector.tensor_mul(msq, mu[:, 0:1], mu[:, 0:1])
                var = spool.tile([1, 1], F32, name="var", tag="var")
                nc.vector.tensor_sub(var, mu[:, 1:2], msq)
                stdg = spool.tile([1, 1], F32, name="stdg", tag="stdg")
                nc.scalar.activation(
                    out=stdg, in_=var, func=AF.Sqrt, bias=eps1, scale=1.0
                )
                rstdg = spool.tile([1, 1], F32, name="rstdg", tag="rstdg")
                nc.vector.reciprocal(out=rstdg, in_=stdg)

                # rstd*||z-m||_r = sqrt((q - 2m*s)*rstd^2 + 256*m^2*rstd^2)
                t2m = spool.tile([1, 1], F32, name="t2m", tag="t2m")
                nc.vector.tensor_scalar_mul(t2m, mu[:, 0:1], 2.0)
                rstd2 = spool.tile([1, 1], F32, name="rstd2", tag="rstd2")
                nc.vector.tensor_mul(rstd2, rstdg, rstdg)
                bias2 = spool.tile([1, 1], F32, name="bias2", tag="bias2")
                nc.vector.tensor_mul(bias2, msq, rstd2)
                nc.vector.tensor_scalar_mul(bias2, bias2, float(OUT))
                cvec = spool.tile([1, R], F32, name="cvec", tag="cvec")
                nc.vector.tensor_scalar(
                    out=cvec, in0=s_sb, scalar1=t2m, scalar2=None, op0=ALU.mult
                )
                nc.vector.tensor_sub(cvec, q_sb, cvec)
                nc.scalar.activation(
                    out=cvec, in_=cvec, func=AF.Sqrt, bias=bias2, scale=rstd2
                )
                nc.vector.tensor_scalar_add(cvec, cvec, 1e-8)
                nc.vector.reciprocal(out=cvec, in_=cvec)
                nc.vector.tensor_scalar(
                    out=cvec, in0=cvec, scalar1=rstdg, scalar2=None, op0=ALU.mult
                )
                # broadcast row-scale and global mean across partitions
                sb_s = spool.tile([128, R], F32, name="sb_s", tag="sb_s")
                for n0, n1 in ((0, 512), (512, R)):
                    pb = psm.tile([128, 512], F32, name="pb", tag="psm")
                    nc.tensor.matmul(
                        pb[:, 0 : n1 - n0], ones_row, cvec[:, n0:n1],
                        start=True, stop=True,
                    )
                    nc.vector.tensor_copy(sb_s[:, n0:n1], pb[:, 0 : n1 - n0])
                pmc = psm.tile([128, 1], F32, name="pmc", tag="psm")
                nc.tensor.matmul(pmc, ones_row, mu[:, 0:1], start=True, stop=True)
                m_col = spool.tile([128, 1], F32, name="m_col", tag="m_col")
                nc.vector.tensor_copy(m_col, pmc)
                # z_final = (z - m) * s_b ; write out
                for mh in range(2):
                    nc.vector.tensor_scalar(
                        out=z_sb[:, mh, :], in0=z_sb[:, mh, :],
                        scalar1=m_col, scalar2=None, op0=ALU.subtract,
                    )
                    nc.vector.tensor_mul(z_sb[:, mh, :], z_sb[:, mh, :], sb_s)
                    nc.scalar.dma_start(out=zout[g][mh, :, :], in_=z_sb[:, mh, :])

    nc.compile()
    return nc


def _get_program():
    if _PROGRAM["nc"] is None:
        _PROGRAM["nc"] = _build_program()
    return _PROGRAM["nc"]


def _shard_inputs(inputs):
    f32 = np.float32
    bf16 = ml_dtypes.bfloat16
    x_all = np.concatenate(
        [
            np.asarray(inputs["drug_feats"], f32),
            np.asarray(inputs["target_feats"], f32),
            np.asarray(inputs["disease_feats"], f32),
        ],
        axis=0,
    )
    adjs = {
        "d": np.asarray(inputs["drug_adjs"], f32),
        "t": np.asarray(inputs["target_adjs"], f32),
    }
    # transposed row-shards in fp8e4m3 (A-side quantization noise averages
    # out; X stays bf16), zero-padded, partition-major tiling [p, ktile, f]
    f8np = mybir.dt.np(F8)
    adjT_shards = {}
    for g in ("d", "t"):
        for lay in (0, 1):
            at = np.ascontiguousarray(adjs[g][lay].T)  # [N, N] col r-major
            per_core = []
            for c in range(N_CORES):
                r0 = c * R
                r1 = min(r0 + R, N_NODES)
                sh = np.zeros((PAD_N, R), dtype=f8np)
                sh[:N_NODES, : r1 - r0] = at[:, r0:r1].astype(f8np)
                sh = np.ascontiguousarray(
                    sh.reshape(NT, 128, R).transpose(1, 0, 2)
                )  # [128, NT, R]
                per_core.append(sh)
            adjT_shards[(g, lay)] = per_core

    aw = {
        0: ("dA", inputs),
        1: ("tA", inputs),
        2: ("sA", inputs),
    }

    w1cat = (
        np.concatenate(
            [np.asarray(inputs["dG_W1"], f32), np.asarray(inputs["tG_W1"], f32)],
            axis=1,
        )
        * np.float32(0.5)
    ).astype(bf16)
    w2d = np.asarray(inputs["dG_W2"], f32) * np.float32(0.5)
    w2t = np.asarray(inputs["tG_W2"], f32) * np.float32(0.5)
    pa_arr = np.array(
        [[np.asarray(inputs["dG_a"], f32)[0], np.asarray(inputs["tG_a"], f32)[0]]], f32
    )

    in_maps = []
    for c in range(N_CORES):
        ty = SEG_TYPE[c]
        pref = aw[ty][0]
        off, v = SEG_OFF[c], SEG_VALID[c]
        xsegT = np.zeros((FEAT, SEG), bf16)
        xsegT[:, :v] = x_all[off : off + v].T.astype(bf16)
        m = {
            "xsegT": xsegT,
            "aWT": np.ascontiguousarray(np.asarray(inputs[pref + "_W"], f32).T).astype(bf16),
            "aWrT": np.ascontiguousarray(np.asarray(inputs[pref + "_Wr"], f32).T).astype(bf16),
            "ab": np.asarray(inputs[pref + "_b"], f32).reshape(1, HID),
            "ag": np.asarray(inputs[pref + "_g"], f32).reshape(1, HID),
            "abeta": np.asarray(inputs[pref + "_beta"], f32).reshape(1, HID),
            "abr": np.asarray(inputs[pref + "_br"], f32).reshape(1, HID),
            "ad0": adjT_shards[("d", 0)][c],
            "ad1": adjT_shards[("d", 1)][c],
            "at0": adjT_shards[("t", 0)][c],
            "at1": adjT_shards[("t", 1)][c],
            "w1cat": w1cat,
            "w2d": w2d,
            "w2t": w2t,
            "pa": pa_arr,
        }
        in_maps.append(m)
    return in_maps


def run_on_device(inputs, trace=False):
    nc = _get_program()
    in_maps = _shard_inputs(inputs)
    res = bass_utils.run_bass_kernel_spmd(
        nc, in_maps, core_ids=list(range(N_CORES)), trace=trace
    )
    # assemble
    def shard_rows(core, name):
        return (
            np.asarray(res.results[core][name]).reshape(2 * 128, R).T.astype(np.float32)
        )

    zd0 = shard_rows(0, "zd")
    z_drug = np.ascontiguousarray(zd0[:N_DRUG])
    zt0 = shard_rows(0, "zt")
    zt1 = shard_rows(1, "zt")
    zt2 = shard_rows(2, "zt")
    z_target = np.ascontiguousarray(
        np.concatenate([zt0[N_DRUG:R], zt1, zt2[: N_DRUG + N_TARGET - 2 * R]], axis=0)
    )
    return (z_drug, z_target), res


def kernel(**inputs):
    out, _ = run_on_device(inputs, trace=False)
    return out
